# revision 3
# baseline (speedup 1.0000x reference)
"""MultiHeadAttention TRN2 kernel v2 — (batch, head-half) sharding.

Core c owns batch b=c//2 and heads j*8..j*8+8 where j=c%2 (feature cols
cs = j*512:(j+1)*512 of Wq/Wk/Wv, rows cs of Wo). Each core computes its
batch+heads' projections, attention, and a half-contraction partial of the
output projection; the host sums the 2 partials per batch.

Device math (per core), matmuls bf16 with f32 PSUM accumulation:
  qT/kT = (Wq_c^T x_b^T + bq_c)     feature-major [128 hd-pair, 2048]  x4 tiles
  v_aug = [x_b^T^T Wv_c | 1]        position-major [128 kv, 8 heads x 65]
                                    (bv dropped; bv@Wo added on host)
  scoresT[kv, q] = kT^T qT          per (h, qc512); exp via ACT, scale=1/8
  attn[q, d|den] = sum_kv expT[kv, q]^T v_aug[kv, d|1]   [128 q, 65] tiles
                                    (col 64 = softmax denominator)
  attn_n = attn * recip(den)        DVE per-partition scalar mul -> bf16
  attnT  = PE-transpose(attn_n)     [128 hd-pair, q] for outproj lhsT
  out_partial[q, e] = attnT^T Wo_c  f32, accumulated over 4 hd-pairs

x inputs are DMA'd per projection tile (no persistent x staging), so SBUF
holds deep exp lookahead instead; chunk order is head-pair-major so the
remaining k/q projections spread smoothly between attention chunks.
"""

import sys

sys.path.insert(0, "/opt/trn_rl_repo")

import numpy as np
import ml_dtypes

import concourse.bass as bass
from concourse import bacc
import concourse.mybir as mybir
from concourse.tile import TileContext
from concourse.bass_utils import run_bass_kernel_spmd

BF16 = mybir.dt.bfloat16
F32 = mybir.dt.float32
AF = mybir.ActivationFunctionType

EMBED = 1024
HEADS = 16
HEAD_DIM = 64
N_CORES = 8
DC = 512  # feature columns per core (8 heads * 64)
NEC = 8  # contraction chunks of 128 over EMBED
NHT = 4  # hd-pair tiles of 128 per core
NKV = 16  # kv tiles of 128 (S=2048)
NQC = 4  # q chunks of 512
NSUB = 4  # 128-wide subtiles per q chunk


def build_nc(B=4, S=2048, reps=1):
    assert S == 2048
    nc = bacc.Bacc("TRN2", target_bir_lowering=False)

    xq_d = nc.declare_dram_parameter("xq", [EMBED, S], BF16, isOutput=False)
    xk_d = nc.declare_dram_parameter("xk", [EMBED, S], BF16, isOutput=False)
    xv_d = nc.declare_dram_parameter("xv", [EMBED, S], BF16, isOutput=False)
    wq_d = nc.declare_dram_parameter("wq", [128, NEC * DC], BF16, isOutput=False)
    wk_d = nc.declare_dram_parameter("wk", [128, NEC * DC], BF16, isOutput=False)
    wv_d = nc.declare_dram_parameter("wv", [128, NEC * DC], BF16, isOutput=False)
    bq_d = nc.declare_dram_parameter("bq", [128, NHT], F32, isOutput=False)
    bk_d = nc.declare_dram_parameter("bk", [128, NHT], F32, isOutput=False)
    wo_d = nc.declare_dram_parameter("wo", [128, NHT * EMBED], BF16, isOutput=False)
    id_d = nc.declare_dram_parameter("ident", [128, 128], BF16, isOutput=False)
    out_d = nc.declare_dram_parameter("out", [S, EMBED], F32, isOutput=True)

    with TileContext(nc) as tc:
        with (
            tc.tile_pool(name="const", bufs=1) as cpool,
            tc.tile_pool(name="big", bufs=1) as big,
            tc.tile_pool(name="xin", bufs=16) as xin,
            tc.tile_pool(name="xvin", bufs=4) as xvin,
            tc.tile_pool(name="expp", bufs=12) as expp,
            tc.tile_pool(name="ev", bufs=8) as evp,
            tc.tile_pool(name="st", bufs=4) as stp,
            tc.tile_pool(name="ot", bufs=2) as otp,
            tc.tile_pool(name="ps", bufs=1, space="PSUM") as ps,
        ):
            def emit_all():
                # --- weights / constants ---
                wv_sb = cpool.tile([128, NEC * DC], BF16, tag="wv")
                wk_sb = cpool.tile([128, NEC * DC], BF16, tag="wk")
                wq_sb = cpool.tile([128, NEC * DC], BF16, tag="wq")
                wo_sb = cpool.tile([128, NHT * EMBED], BF16, tag="wo")
                bq_sb = cpool.tile([128, NHT], F32, tag="bq")
                bk_sb = cpool.tile([128, NHT], F32, tag="bk")
                id_sb = cpool.tile([128, 128], BF16, tag="id")
                nc.gpsimd.dma_start(out=wv_sb[:], in_=wv_d[:])
                nc.gpsimd.dma_start(out=wk_sb[:], in_=wk_d[:])
                nc.gpsimd.dma_start(out=wq_sb[:], in_=wq_d[:])
                nc.gpsimd.dma_start(out=bk_sb[:], in_=bk_d[:])
                nc.gpsimd.dma_start(out=bq_sb[:], in_=bq_d[:])
                nc.gpsimd.dma_start(out=id_sb[:], in_=id_d[:])

                # persistent per-core SBUF state
                qT_sb = big.tile([128, NHT * S], BF16, tag="qT")  # tile t: heads 2t,2t+1
                kT_sb = big.tile([128, NHT * S], BF16, tag="kT")
                v_sb = big.tile([128, NKV * 8 * 65], BF16, tag="v")  # [kvt][h][65]
                attnT_sb = big.tile([128, NHT * S], BF16, tag="attnT")
                nc.vector.memset(v_sb[:], 1.0)  # ones col (idx 64) per 65-block

                # --- projections ---
                def dma_x(which, src_d, eng):
                    ts = []
                    for ec in range(NEC):
                        t = xin.tile([128, S], BF16, tag="xin", name=f"x{which}{ec}")
                        eng.dma_start(
                            out=t[:], in_=src_d[ec * 128 : (ec + 1) * 128, :]
                        )
                        ts.append(t)
                    return ts

                def qk_proj_tile(xts, wsb, bsb, dst, hc, rc):
                    # dst[:, hc*S + rc*512 : +512] = W_hc^T x[:, rc*512:+512] + b
                    pt = ps.tile([128, 512], F32, tag="misc", bufs=2, name="pt")
                    for ec in range(NEC):
                        nc.tensor.matmul(
                            pt[:],
                            wsb[:, ec * DC + hc * 128 : ec * DC + (hc + 1) * 128],
                            xts[ec][:, rc * 512 : (rc + 1) * 512],
                            start=(ec == 0),
                            stop=(ec == NEC - 1),
                        )
                    nc.vector.tensor_scalar_add(
                        dst[:, hc * S + rc * 512 : hc * S + (rc + 1) * 512],
                        pt[:],
                        bsb[:, hc : hc + 1],
                    )

                def v_proj_tile(kvt, half):
                    # v rows kvt*128..+128, heads half*4..half*4+4 (256 cols)
                    # half0 loads x_v via gpsimd SWDGE (idle at start);
                    # half1 re-loads on SP (idle during steady state)
                    eng = nc.gpsimd if half == 0 else nc.sync
                    xt = xvin.tile([128, NEC * 128], BF16, tag="xv", name=f"xv{kvt}")
                    eng.dma_start(
                        out=xt[:],
                        in_=xv_d[:, kvt * 128 : (kvt + 1) * 128].rearrange(
                            "(a p) c -> p a c", p=128
                        ),
                    )
                    pv = ps.tile([128, 256], F32, tag="misc", bufs=2, name="pv")
                    for ec in range(NEC):
                        nc.tensor.matmul(
                            pv[:],
                            xt[:, ec * 128 : (ec + 1) * 128],
                            wv_sb[:, ec * DC + half * 256 : ec * DC + half * 256 + 256],
                            start=(ec == 0),
                            stop=(ec == NEC - 1),
                        )
                    base = kvt * 8 * 65 + half * 4 * 65
                    for h in range(4):
                        nc.vector.tensor_copy(
                            v_sb[:, base + h * 65 : base + h * 65 + 64],
                            pv[:, h * 64 : (h + 1) * 64],
                        )

                # --- attention ---
                def emit_scores(h, qc):
                    # 8 exp tiles, each [128 kv (pair of kv tiles), 2x512 q]
                    hc, dr = h // 2, (h % 2) * 64
                    q0 = qc * 512
                    ets = []
                    for kp in range(NKV // 2):
                        sps = ps.tile([128, 1024], F32, tag="sps", bufs=2, name="sps")
                        for j in range(2):
                            kvt = kp * 2 + j
                            nc.tensor.matmul(
                                sps[:, j * 512 : (j + 1) * 512],
                                kT_sb[dr : dr + 64, hc * S + kvt * 128 : hc * S + (kvt + 1) * 128],
                                qT_sb[dr : dr + 64, hc * S + q0 : hc * S + q0 + 512],
                                start=True,
                                stop=True,
                            )
                        e_t = expp.tile([128, 1024], BF16, tag="expp", name="et")
                        nc.scalar.activation(e_t[:], sps[:], AF.Exp, scale=0.125)
                        ets.append(e_t)
                    return ets

                def emit_attnv(h, qc, ets, att_st):
                    # apt[128 q, 65] per 128-q subtile; col 64 = denom
                    hh = h % 2
                    for sub in range(NSUB):
                        apt = ps.tile([128, 65], F32, tag="apt", bufs=2, name="apt")
                        for kvt in range(NKV):
                            et = ets[kvt // 2]
                            ecol = (kvt % 2) * 512 + sub * 128
                            vbase = kvt * 8 * 65 + h * 65
                            nc.tensor.matmul(
                                apt[:],
                                et[:, ecol : ecol + 128],
                                v_sb[:, vbase : vbase + 65],
                                start=(kvt == 0),
                                stop=(kvt == NKV - 1),
                            )
                        rec = evp.tile([128, 1], F32, tag="rec", name="rec")
                        nc.vector.reciprocal(rec[:], apt[:, 64:65])
                        nc.vector.tensor_scalar_mul(
                            att_st[:, sub * 128 + hh * 64 : sub * 128 + hh * 64 + 64],
                            apt[:, 0:64],
                            rec[:, 0:1],
                        )

                def emit_transpose(p, qc, att_st):
                    # att_st [128 q, 4*128] -> attnT tile p cols qc*512..
                    for sub in range(NSUB):
                        tps = ps.tile([128, 128], BF16, tag="misc", bufs=2, name="tps")
                        nc.tensor.transpose(
                            tps[:], att_st[:, sub * 128 : (sub + 1) * 128], id_sb[:]
                        )
                        nc.vector.tensor_copy(
                            attnT_sb[
                                :, p * S + qc * 512 + sub * 128 : p * S + qc * 512 + (sub + 1) * 128
                            ],
                            tps[:],
                        )

                def emit_outproj_sub(qc, sub):
                    if True:
                        c0 = qc * 512 + sub * 128
                        ot = otp.tile([128, EMBED], F32, tag="ot", name="ot")
                        for en in range(2):
                            po = ps.tile([128, 512], F32, tag="misc", bufs=2, name="po")
                            for p in range(NHT):
                                nc.tensor.matmul(
                                    po[:],
                                    attnT_sb[:, p * S + c0 : p * S + c0 + 128],
                                    wo_sb[:, p * EMBED + en * 512 : p * EMBED + (en + 1) * 512],
                                    start=(p == 0),
                                    stop=(p == NHT - 1),
                                )
                            nc.vector.tensor_copy(ot[:, en * 512 : (en + 1) * 512], po[:])
                        nc.sync.dma_start(out=out_d[c0 : c0 + 128, :], in_=ot[:])

                # ---------------- schedule ----------------
                # Prologue: xk on SP, xq on the ACT DMA channel (done before
                # exp work exists), k proj pair0 + first q tile + first scores
                # so ACT starts ~16us in; then v proj heads 0-3 (xv via
                # gpsimd SWDGE); remaining projections spread as fillers.
                xk = dma_x("k", xk_d, nc.sync)
                xq = dma_x("q", xq_d, nc.scalar)
                for rc in range(4):
                    qk_proj_tile(xk, wk_sb, bk_sb, kT_sb, 0, rc)
                qk_proj_tile(xq, wq_sb, bq_sb, qT_sb, 0, 0)

                pend = []
                att_st = {}

                def start_chunk(h, qc):
                    p = h // 2
                    if (p, qc) not in att_st:
                        att_st[(p, qc)] = stp.tile(
                            [128, NSUB * 128], BF16, tag="att", name=f"att{p}_{qc}"
                        )
                    pend.append((h, qc, emit_scores(h, qc)))

                def finish_chunk():
                    h, qc, ets = pend.pop(0)
                    p = h // 2
                    emit_attnv(h, qc, ets, att_st[(p, qc)])
                    if h % 2 == 1:
                        emit_transpose(p, qc, att_st.pop((p, qc)))
                    return h, qc

                # pair-major chunk order
                order = [
                    (2 * p + hh, qc)
                    for p in range(NHT)
                    for qc in range(NQC)
                    for hh in range(2)
                ]
                fillers = {}

                def addf(i, f):
                    fillers.setdefault(i, []).append(f)

                # q proj hc0 rc1..3: needed by (0,qc) at i=2qc
                for rcq in (1, 2, 3):
                    addf(2 * rcq - 1, lambda rc=rcq: qk_proj_tile(xq, wq_sb, bq_sb, qT_sb, 0, rc))
                # v proj heads 4-7: needed by pair 2 at i=16
                for n, kvt in enumerate(range(NKV)):
                    addf(1 + (n * 14) // NKV, lambda kvt=kvt: v_proj_tile(kvt, 1))
                # k/q proj pairs 1..3
                for p in range(1, NHT):
                    i0 = 8 * (p - 1)
                    for n in range(4):
                        addf(i0 + 2 * n, lambda p=p, rc=n: qk_proj_tile(xk, wk_sb, bk_sb, kT_sb, p, rc))
                    addf(i0 + 7, lambda p=p: qk_proj_tile(xq, wq_sb, bq_sb, qT_sb, p, 0))
                    for rcq in (1, 2, 3):
                        addf(8 * p + 2 * rcq - 1, lambda p=p, rc=rcq: qk_proj_tile(xq, wq_sb, bq_sb, qT_sb, p, rc))

                start_chunk(*order[0])
                for kvt in range(NKV):
                    v_proj_tile(kvt, 0)
                nc.gpsimd.dma_start(out=wo_sb[:], in_=wo_d[:])
                start_chunk(*order[1])
                op_queue = []  # (qc, sub) outproj work, spread between chunks
                for i in range(2, len(order)):
                    for j in sorted(k for k in fillers if k <= i - 1):
                        for f in fillers.pop(j):
                            f()
                    if op_queue:
                        emit_outproj_sub(*op_queue.pop(0))
                    start_chunk(*order[i])
                    h, qc = finish_chunk()
                    # pair 3 chunk (h=7, qc) completing means all pairs have
                    # these q rows done -> outproj them (spread over
                    # subsequent iterations to keep ACT fed)
                    if h == 7:
                        op_queue.extend((qc, sub) for sub in range(NSUB))
                    if op_queue:
                        emit_outproj_sub(*op_queue.pop(0))
                for f in [f for i in sorted(fillers) for f in fillers[i]]:
                    f()
                h, qc = finish_chunk()
                if h == 7:
                    op_queue.extend((qc, sub) for sub in range(NSUB))
                h, qc = finish_chunk()
                op_queue.extend((qc, sub) for sub in range(NSUB))
                for qc, sub in op_queue:
                    emit_outproj_sub(qc, sub)

            if reps == 1:
                emit_all()
            else:
                with tc.For_i(0, reps):
                    emit_all()

    nc.finalize()
    return nc


_NC_CACHE = {}


def get_nc(B=4, S=2048, reps=1):
    key = (B, S, reps)
    if key not in _NC_CACHE:
        _NC_CACHE[key] = build_nc(B, S, reps)
    return _NC_CACHE[key]


def make_in_maps(value, key, query, Wv, bv, Wk, bk, Wq, bq, Wo, bo, B, S):
    bf = ml_dtypes.bfloat16
    ident = np.eye(128, dtype=bf)

    def packw(W, cs):
        return np.ascontiguousarray(
            W[:, cs].astype(bf).reshape(NEC, 128, DC).transpose(1, 0, 2).reshape(128, NEC * DC)
        )

    def packb(b_, cs):
        return np.ascontiguousarray(b_[cs].astype(np.float32).reshape(NHT, 128).T)

    in_maps = []
    for c in range(N_CORES):
        b, j = c // 2, c % 2
        cs = slice(j * DC, (j + 1) * DC)
        in_maps.append(
            {
                "xq": np.ascontiguousarray(query[b].astype(bf).T),
                "xk": np.ascontiguousarray(key[b].astype(bf).T),
                "xv": np.ascontiguousarray(value[b].astype(bf).T),
                "wq": packw(Wq, cs),
                "wk": packw(Wk, cs),
                "wv": packw(Wv, cs),
                "bq": packb(bq, cs),
                "bk": packb(bk, cs),
                "wo": np.ascontiguousarray(
                    Wo[cs, :]
                    .astype(bf)
                    .reshape(NHT, 128, EMBED)
                    .transpose(1, 0, 2)
                    .reshape(128, NHT * EMBED)
                ),
                "ident": ident,
            }
        )
    return in_maps


def finish(results, Wv, bv, Wo, bo, B, S):
    const = (bv.astype(np.float32) @ Wo.astype(np.float32) + bo.astype(np.float32))[
        None, :
    ]
    outs = []
    for b in range(B):
        acc = results[2 * b]["out"].astype(np.float32) + results[2 * b + 1]["out"]
        outs.append(acc + const)
    return np.stack(outs, axis=0)


def kernel(value, key, query, Wv, bv, Wk, bk, Wq, bq, Wo, bo):
    B, S, _ = query.shape
    nc = get_nc(B, S)
    in_maps = make_in_maps(value, key, query, Wv, bv, Wk, bk, Wq, bq, Wo, bo, B, S)
    res = run_bass_kernel_spmd(nc, in_maps, list(range(N_CORES)))
    return finish(res.results, Wv, bv, Wo, bo, B, S)


# revision 4
# speedup vs baseline: 1.0209x; 1.0209x over previous
"""MultiHeadAttention TRN2 kernel v2 — (batch, head-half) sharding.

Core c owns batch b=c//2 and heads j*8..j*8+8 where j=c%2 (feature cols
cs = j*512:(j+1)*512 of Wq/Wk/Wv, rows cs of Wo). Each core computes its
batch+heads' projections, attention, and a half-contraction partial of the
output projection; the host sums the 2 partials per batch.

Device math (per core), matmuls bf16 with f32 PSUM accumulation:
  qT/kT = (Wq_c^T x_b^T + bq_c)     feature-major [128 hd-pair, 2048]  x4 tiles
  v_aug = [x_b^T^T Wv_c | 1]        position-major [128 kv, 8 heads x 65]
                                    (bv dropped; bv@Wo added on host)
  scoresT[kv, q] = kT^T qT          per (h, qc512); exp via ACT, scale=1/8
  attn[q, d|den] = sum_kv expT[kv, q]^T v_aug[kv, d|1]   [128 q, 65] tiles
                                    (col 64 = softmax denominator)
  attn_n = attn * recip(den)        DVE per-partition scalar mul -> bf16
  attnT  = PE-transpose(attn_n)     [128 hd-pair, q] for outproj lhsT
  out_partial[q, e] = attnT^T Wo_c  f32, accumulated over 4 hd-pairs

x inputs are DMA'd per projection tile (no persistent x staging), so SBUF
holds deep exp lookahead instead; chunk order is head-pair-major so the
remaining k/q projections spread smoothly between attention chunks.
"""

import sys

sys.path.insert(0, "/opt/trn_rl_repo")

import numpy as np
import ml_dtypes

import concourse.bass as bass
from concourse import bacc
import concourse.mybir as mybir
from concourse.tile import TileContext
from concourse.bass_utils import run_bass_kernel_spmd

BF16 = mybir.dt.bfloat16
F32 = mybir.dt.float32
AF = mybir.ActivationFunctionType

EMBED = 1024
HEADS = 16
HEAD_DIM = 64
N_CORES = 8
DC = 512  # feature columns per core (8 heads * 64)
NEC = 8  # contraction chunks of 128 over EMBED
NHT = 4  # hd-pair tiles of 128 per core
NKV = 16  # kv tiles of 128 (S=2048)
NQC = 4  # q chunks of 512
NSUB = 4  # 128-wide subtiles per q chunk


def build_nc(B=4, S=2048, reps=1):
    assert S == 2048
    nc = bacc.Bacc("TRN2", target_bir_lowering=False)

    xq_d = nc.declare_dram_parameter("xq", [EMBED, S], BF16, isOutput=False)
    xk_d = nc.declare_dram_parameter("xk", [EMBED, S], BF16, isOutput=False)
    xv_d = nc.declare_dram_parameter("xv", [EMBED, S], BF16, isOutput=False)
    wq_d = nc.declare_dram_parameter("wq", [128, NEC * DC], BF16, isOutput=False)
    wk_d = nc.declare_dram_parameter("wk", [128, NEC * DC], BF16, isOutput=False)
    wv_d = nc.declare_dram_parameter("wv", [128, NEC * DC], BF16, isOutput=False)
    bq_d = nc.declare_dram_parameter("bq", [128, NHT], F32, isOutput=False)
    bk_d = nc.declare_dram_parameter("bk", [128, NHT], F32, isOutput=False)
    wo_d = nc.declare_dram_parameter("wo", [128, NHT * EMBED], BF16, isOutput=False)
    id_d = nc.declare_dram_parameter("ident", [128, 128], BF16, isOutput=False)
    out_d = nc.declare_dram_parameter("out", [S, EMBED], F32, isOutput=True)

    with TileContext(nc) as tc:
        with (
            tc.tile_pool(name="const", bufs=1) as cpool,
            tc.tile_pool(name="big", bufs=1) as big,
            tc.tile_pool(name="xin", bufs=64) as xin,
            tc.tile_pool(name="xvin", bufs=4) as xvin,
            tc.tile_pool(name="expp", bufs=12) as expp,
            tc.tile_pool(name="ev", bufs=8) as evp,
            tc.tile_pool(name="st", bufs=4) as stp,
            tc.tile_pool(name="ot", bufs=2) as otp,
            tc.tile_pool(name="ps", bufs=1, space="PSUM") as ps,
        ):
            def emit_all():
                # --- weights / constants ---
                wv_sb = cpool.tile([128, NEC * DC], BF16, tag="wv")
                wk_sb = cpool.tile([128, NEC * DC], BF16, tag="wk")
                wq_sb = cpool.tile([128, NEC * DC], BF16, tag="wq")
                wo_sb = cpool.tile([128, NHT * EMBED], BF16, tag="wo")
                bq_sb = cpool.tile([128, NHT], F32, tag="bq")
                bk_sb = cpool.tile([128, NHT], F32, tag="bk")
                id_sb = cpool.tile([128, 128], BF16, tag="id")
                nc.gpsimd.dma_start(out=wv_sb[:], in_=wv_d[:])
                nc.gpsimd.dma_start(out=wk_sb[:], in_=wk_d[:])
                nc.gpsimd.dma_start(out=wq_sb[:], in_=wq_d[:])
                nc.gpsimd.dma_start(out=bk_sb[:], in_=bk_d[:])
                nc.gpsimd.dma_start(out=bq_sb[:], in_=bq_d[:])
                nc.gpsimd.dma_start(out=id_sb[:], in_=id_d[:])

                # persistent per-core SBUF state
                qT_sb = big.tile([128, NHT * S], BF16, tag="qT")  # tile t: heads 2t,2t+1
                kT_sb = big.tile([128, NHT * S], BF16, tag="kT")
                v_sb = big.tile([128, NKV * 8 * 65], BF16, tag="v")  # [kvt][h][65]
                attnT_sb = big.tile([128, NHT * S], BF16, tag="attnT")

                # --- projections ---
                def dma_x_rc(which, src_d, eng, ts, rc):
                    # stage x[:, rc*512:+512] as 8 [128,512] ec tiles
                    for ec in range(NEC):
                        t = xin.tile([128, 512], BF16, tag="xin", name=f"x{which}{ec}_{rc}")
                        eng.dma_start(
                            out=t[:],
                            in_=src_d[
                                ec * 128 : (ec + 1) * 128, rc * 512 : (rc + 1) * 512
                            ],
                        )
                        ts[(ec, rc)] = t

                def qk_proj_tile(xts, wsb, bsb, dst, hc, rc):
                    # dst[:, hc*S + rc*512 : +512] = W_hc^T x[:, rc*512:+512] + b
                    pt = ps.tile([128, 512], F32, tag="misc", bufs=2, name="pt")
                    for ec in range(NEC):
                        nc.tensor.matmul(
                            pt[:],
                            wsb[:, ec * DC + hc * 128 : ec * DC + (hc + 1) * 128],
                            xts[(ec, rc)][:],
                            start=(ec == 0),
                            stop=(ec == NEC - 1),
                        )
                    nc.vector.tensor_scalar_add(
                        dst[:, hc * S + rc * 512 : hc * S + (rc + 1) * 512],
                        pt[:],
                        bsb[:, hc : hc + 1],
                    )

                def v_proj_tile(kvt, half):
                    # v rows kvt*128..+128, heads half*4..half*4+4 (256 cols)
                    # half0 loads x_v via gpsimd SWDGE (idle at start);
                    # half1 re-loads on SP (idle during steady state)
                    eng = nc.gpsimd if half == 0 else nc.sync
                    xt = xvin.tile([128, NEC * 128], BF16, tag="xv", name=f"xv{kvt}")
                    eng.dma_start(
                        out=xt[:],
                        in_=xv_d[:, kvt * 128 : (kvt + 1) * 128].rearrange(
                            "(a p) c -> p a c", p=128
                        ),
                    )
                    pv = ps.tile([128, 256], F32, tag="misc", bufs=2, name="pv")
                    for ec in range(NEC):
                        nc.tensor.matmul(
                            pv[:],
                            xt[:, ec * 128 : (ec + 1) * 128],
                            wv_sb[:, ec * DC + half * 256 : ec * DC + half * 256 + 256],
                            start=(ec == 0),
                            stop=(ec == NEC - 1),
                        )
                    base = kvt * 8 * 65 + half * 4 * 65
                    for h in range(4):
                        nc.vector.tensor_copy(
                            v_sb[:, base + h * 65 : base + h * 65 + 64],
                            pv[:, h * 64 : (h + 1) * 64],
                        )

                # --- attention ---
                def emit_scores(h, qc):
                    # 8 exp tiles, each [128 kv (pair of kv tiles), 2x512 q]
                    hc, dr = h // 2, (h % 2) * 64
                    q0 = qc * 512
                    ets = []
                    for kp in range(NKV // 2):
                        sps = ps.tile([128, 1024], F32, tag="sps", bufs=2, name="sps")
                        for j in range(2):
                            kvt = kp * 2 + j
                            nc.tensor.matmul(
                                sps[:, j * 512 : (j + 1) * 512],
                                kT_sb[dr : dr + 64, hc * S + kvt * 128 : hc * S + (kvt + 1) * 128],
                                qT_sb[dr : dr + 64, hc * S + q0 : hc * S + q0 + 512],
                                start=True,
                                stop=True,
                            )
                        e_t = expp.tile([128, 1024], BF16, tag="expp", name="et")
                        nc.scalar.activation(e_t[:], sps[:], AF.Exp, scale=0.125)
                        ets.append(e_t)
                    return ets

                def emit_attnv(h, qc, ets, att_st):
                    # apt[128 q, 65] per 128-q subtile; col 64 = denom
                    hh = h % 2
                    for sub in range(NSUB):
                        apt = ps.tile([128, 65], F32, tag="apt", bufs=2, name="apt")
                        for kvt in range(NKV):
                            et = ets[kvt // 2]
                            ecol = (kvt % 2) * 512 + sub * 128
                            vbase = kvt * 8 * 65 + h * 65
                            nc.tensor.matmul(
                                apt[:],
                                et[:, ecol : ecol + 128],
                                v_sb[:, vbase : vbase + 65],
                                start=(kvt == 0),
                                stop=(kvt == NKV - 1),
                            )
                        rec = evp.tile([128, 1], F32, tag="rec", name="rec")
                        nc.vector.reciprocal(rec[:], apt[:, 64:65])
                        nc.vector.tensor_scalar_mul(
                            att_st[:, sub * 128 + hh * 64 : sub * 128 + hh * 64 + 64],
                            apt[:, 0:64],
                            rec[:, 0:1],
                        )

                def emit_transpose(p, qc, att_st):
                    # att_st [128 q, 4*128] -> attnT tile p cols qc*512..
                    for sub in range(NSUB):
                        tps = ps.tile([128, 128], BF16, tag="misc", bufs=2, name="tps")
                        nc.tensor.transpose(
                            tps[:], att_st[:, sub * 128 : (sub + 1) * 128], id_sb[:]
                        )
                        nc.vector.tensor_copy(
                            attnT_sb[
                                :, p * S + qc * 512 + sub * 128 : p * S + qc * 512 + (sub + 1) * 128
                            ],
                            tps[:],
                        )

                def emit_outproj_sub(qc, sub):
                    if True:
                        c0 = qc * 512 + sub * 128
                        ot = otp.tile([128, EMBED], F32, tag="ot", name="ot")
                        for en in range(2):
                            po = ps.tile([128, 512], F32, tag="misc", bufs=2, name="po")
                            for p in range(NHT):
                                nc.tensor.matmul(
                                    po[:],
                                    attnT_sb[:, p * S + c0 : p * S + c0 + 128],
                                    wo_sb[:, p * EMBED + en * 512 : p * EMBED + (en + 1) * 512],
                                    start=(p == 0),
                                    stop=(p == NHT - 1),
                                )
                            nc.vector.tensor_copy(ot[:, en * 512 : (en + 1) * 512], po[:])
                        nc.sync.dma_start(out=out_d[c0 : c0 + 128, :], in_=ot[:])

                # ---------------- schedule ----------------
                # Prologue: xk[:, 0:512] on SP and xq[:, 0:512] on the ACT
                # DMA channel arrive in ~3us; k-hc0 tiles are interleaved
                # with the first chunk's score pairs so ACT starts ~7us in.
                # Then v proj heads 0-3 (xv via gpsimd SWDGE); remaining
                # projections spread as fillers.
                xk, xq = {}, {}
                dma_x_rc("k", xk_d, nc.sync, xk, 0)
                dma_x_rc("q", xq_d, nc.scalar, xq, 0)
                for rc in (1, 2, 3):
                    dma_x_rc("k", xk_d, nc.sync, xk, rc)
                for rc in (1, 2, 3):
                    dma_x_rc("q", xq_d, nc.sync, xq, rc)

                pend = []
                att_st = {}

                def start_chunk(h, qc):
                    p = h // 2
                    if (p, qc) not in att_st:
                        att_st[(p, qc)] = stp.tile(
                            [128, NSUB * 128], BF16, tag="att", name=f"att{p}_{qc}"
                        )
                    pend.append((h, qc, emit_scores(h, qc)))

                def finish_chunk():
                    h, qc, ets = pend.pop(0)
                    p = h // 2
                    emit_attnv(h, qc, ets, att_st[(p, qc)])
                    if h % 2 == 1:
                        emit_transpose(p, qc, att_st.pop((p, qc)))
                    return h, qc

                # pair-major chunk order
                order = [
                    (2 * p + hh, qc)
                    for p in range(NHT)
                    for qc in range(NQC)
                    for hh in range(2)
                ]
                fillers = {}

                def addf(i, f):
                    fillers.setdefault(i, []).append(f)

                # q proj hc0 rc1..3: needed by (0,qc) at i=2qc
                for rcq in (1, 2, 3):
                    addf(2 * rcq - 1, lambda rc=rcq: qk_proj_tile(xq, wq_sb, bq_sb, qT_sb, 0, rc))
                # v proj heads 4-7: needed by pair 2 at i=16
                for n, kvt in enumerate(range(NKV)):
                    addf(1 + (n * 14) // NKV, lambda kvt=kvt: v_proj_tile(kvt, 1))
                # k/q proj pairs 1..3
                for p in range(1, NHT):
                    i0 = 8 * (p - 1)
                    for n in range(4):
                        addf(i0 + 2 * n, lambda p=p, rc=n: qk_proj_tile(xk, wk_sb, bk_sb, kT_sb, p, rc))
                    addf(i0 + 7, lambda p=p: qk_proj_tile(xq, wq_sb, bq_sb, qT_sb, p, 0))
                    for rcq in (1, 2, 3):
                        addf(8 * p + 2 * rcq - 1, lambda p=p, rc=rcq: qk_proj_tile(xq, wq_sb, bq_sb, qT_sb, p, rc))

                # chunk (0,0) with k tiles JIT-interleaved before each
                # pair of score matmuls that needs them
                qk_proj_tile(xk, wk_sb, bk_sb, kT_sb, 0, 0)
                qk_proj_tile(xq, wq_sb, bq_sb, qT_sb, 0, 0)
                # ones col (idx 64) per 65-block; deferred so the first k/q
                # bias-adds aren't stuck behind an 8.7us DVE memset
                nc.vector.memset(v_sb[:], 1.0)
                att_st[(0, 0)] = stp.tile(
                    [128, NSUB * 128], BF16, tag="att", name="att0_0"
                )
                ets0 = []
                for rc in range(4):
                    if rc > 0:
                        qk_proj_tile(xk, wk_sb, bk_sb, kT_sb, 0, rc)
                    for kp in (2 * rc, 2 * rc + 1):
                        sps = ps.tile([128, 1024], F32, tag="sps", bufs=2, name="sps")
                        for j in range(2):
                            kvt = kp * 2 + j
                            nc.tensor.matmul(
                                sps[:, j * 512 : (j + 1) * 512],
                                kT_sb[0:64, kvt * 128 : (kvt + 1) * 128],
                                qT_sb[0:64, 0:512],
                                start=True,
                                stop=True,
                            )
                        e_t = expp.tile([128, 1024], BF16, tag="expp", name="et")
                        nc.scalar.activation(e_t[:], sps[:], AF.Exp, scale=0.125)
                        ets0.append(e_t)
                pend.append((0, 0, ets0))
                for kvt in range(NKV // 2):
                    v_proj_tile(kvt, 0)
                start_chunk(*order[1])
                for kvt in range(NKV // 2, NKV):
                    v_proj_tile(kvt, 0)
                nc.gpsimd.dma_start(out=wo_sb[:], in_=wo_d[:])
                op_queue = []  # (qc, sub) outproj work, spread between chunks
                for i in range(2, len(order)):
                    for j in sorted(k for k in fillers if k <= i - 1):
                        for f in fillers.pop(j):
                            f()
                    if op_queue:
                        emit_outproj_sub(*op_queue.pop(0))
                    start_chunk(*order[i])
                    h, qc = finish_chunk()
                    # pair 3 chunk (h=7, qc) completing means all pairs have
                    # these q rows done -> outproj them (spread over
                    # subsequent iterations to keep ACT fed)
                    if h == 7:
                        op_queue.extend((qc, sub) for sub in range(NSUB))
                    if op_queue:
                        emit_outproj_sub(*op_queue.pop(0))
                for f in [f for i in sorted(fillers) for f in fillers[i]]:
                    f()
                h, qc = finish_chunk()
                if h == 7:
                    op_queue.extend((qc, sub) for sub in range(NSUB))
                h, qc = finish_chunk()
                op_queue.extend((qc, sub) for sub in range(NSUB))
                for qc, sub in op_queue:
                    emit_outproj_sub(qc, sub)

            if reps == 1:
                emit_all()
            else:
                with tc.For_i(0, reps):
                    emit_all()

    nc.finalize()
    return nc


_NC_CACHE = {}


def get_nc(B=4, S=2048, reps=1):
    key = (B, S, reps)
    if key not in _NC_CACHE:
        _NC_CACHE[key] = build_nc(B, S, reps)
    return _NC_CACHE[key]


def make_in_maps(value, key, query, Wv, bv, Wk, bk, Wq, bq, Wo, bo, B, S):
    bf = ml_dtypes.bfloat16
    ident = np.eye(128, dtype=bf)

    def packw(W, cs):
        return np.ascontiguousarray(
            W[:, cs].astype(bf).reshape(NEC, 128, DC).transpose(1, 0, 2).reshape(128, NEC * DC)
        )

    def packb(b_, cs):
        return np.ascontiguousarray(b_[cs].astype(np.float32).reshape(NHT, 128).T)

    in_maps = []
    for c in range(N_CORES):
        b, j = c // 2, c % 2
        cs = slice(j * DC, (j + 1) * DC)
        in_maps.append(
            {
                "xq": np.ascontiguousarray(query[b].astype(bf).T),
                "xk": np.ascontiguousarray(key[b].astype(bf).T),
                "xv": np.ascontiguousarray(value[b].astype(bf).T),
                "wq": packw(Wq, cs),
                "wk": packw(Wk, cs),
                "wv": packw(Wv, cs),
                "bq": packb(bq, cs),
                "bk": packb(bk, cs),
                "wo": np.ascontiguousarray(
                    Wo[cs, :]
                    .astype(bf)
                    .reshape(NHT, 128, EMBED)
                    .transpose(1, 0, 2)
                    .reshape(128, NHT * EMBED)
                ),
                "ident": ident,
            }
        )
    return in_maps


def finish(results, Wv, bv, Wo, bo, B, S):
    const = (bv.astype(np.float32) @ Wo.astype(np.float32) + bo.astype(np.float32))[
        None, :
    ]
    outs = []
    for b in range(B):
        acc = results[2 * b]["out"].astype(np.float32) + results[2 * b + 1]["out"]
        outs.append(acc + const)
    return np.stack(outs, axis=0)


def kernel(value, key, query, Wv, bv, Wk, bk, Wq, bq, Wo, bo):
    B, S, _ = query.shape
    nc = get_nc(B, S)
    in_maps = make_in_maps(value, key, query, Wv, bv, Wk, bk, Wq, bq, Wo, bo, B, S)
    res = run_bass_kernel_spmd(nc, in_maps, list(range(N_CORES)))
    return finish(res.results, Wv, bv, Wo, bo, B, S)
